# revision 12
# baseline (speedup 1.0000x reference)
"""Causal self-attention on 8 trn2 NeuronCores.

Sharding: core c -> (batch b = c // 4, head-group g = c % 4). Each core
computes 4 of the 16 heads for one batch element and the corresponding
slice of the output projection; the host sums the 4 partial projections
per batch and adds the constant bias terms (bv @ Wp.T + bp) exactly.

Device-side structure (per core):
- Projections run in fp32 (float32r full-rate PE mode); q/k are evicted
  to bf16, v to bf16.
- Scores s = k^T q run in bf16 with exact causal trimming (bf16 matmuls
  have no small-N penalty), exp runs on the scalar engine straight into
  bf16 e-tiles, and the causal mask is applied post-exp as a bf16
  triangle multiply on the gpsimd engine (PE pays nothing for masking).
- attn@v is reoriented: e is the stationary operand ([keys, q-tile],
  K=128, M=128 fully used) and v streams as moving data (N=65: dh plus
  a ones column that accumulates the softmax denominator l), which
  halves the PE time of this phase vs. the v-stationary orientation.
  y lands as [q-partition, dh | l] and normalization is a per-partition
  tensor_scalar multiply during PSUM eviction (no cross-partition
  broadcast needed).
- PSUM accumulation groups "zero regions" are a full 2 KiB bank, so
  interleaving several open accumulation groups in one bank corrupts
  them. Attention therefore runs as 4 sequential head-streams per
  chunk: a stream's scores+exp burst fills SBUF e-tiles, then its
  attn@v runs one (qtile) accumulation group at a time into that
  head's dedicated PSUM bank.
- y is transposed back to [dh, q] with PE transposes (bf16) for the
  f32r output projection.
- The attention m-loop interleaves projection / output-projection /
  transpose matmuls as filler so the PE never waits on the scalar
  engine's exp latency.
"""

from collections import deque

import numpy as np

import concourse.bass as bass
import concourse.mybir as mybir
import concourse.tile as tile
from concourse.bass_utils import run_bass_kernel_spmd

B = 2
T = 2048
C = 1024
H = 16
DH = 64
NCORES = 8
GROUPS = 4           # head groups (tensor parallel)
HPG = H // GROUPS    # heads per group = 4
DG = HPG * DH        # head-group width = 256
CHUNK = 512          # query-block size
NCHUNK = T // CHUNK  # 4
KO = C // 128        # 8 contraction subtiles for the projections
MT = DG // 128       # 2 partition tiles for qT/kT and wp rows
F32 = mybir.dt.float32
F32R = mybir.dt.float32r
BF16 = mybir.dt.bfloat16


def _patch_tile_drain():
    """This walrus build lowers Drain/NOP to a CTRL with a single sync-wait
    slot; TileContext's kernel-tail drain accumulates one wait per live
    semaphore and fails codegen. Split the waits across single-wait NOPs."""
    import bass_rust
    from concourse.tile import TileContext

    def _drain_and_barrier_split(self, tick_clock, wait_clock):
        probe = self.nc.sync.nop()
        wait_clock.add_sem_waits(
            probe.ins, tile.ScopedClock({None: tick_clock.global_clock})
        )
        waits = list(probe.ins.sync_info.on_wait or [])
        probe.ins.sync_info.on_wait = []
        # distribute the final-value waits across engines; the all-engine
        # barrier below joins them before the semaphore reset
        engines = [self.nc.sync, self.nc.tensor, self.nc.vector,
                   self.nc.scalar, self.nc.gpsimd]
        for i, w in enumerate(waits):
            n = engines[i % len(engines)].nop()
            if n.ins.sync_info is None:
                n.ins.sync_info = bass_rust.SyncInfo(on_wait=[w], on_update=[])
            else:
                n.ins.sync_info.on_wait = [w]
        self.nc.sync.drain()
        self.nc.all_engine_barrier()
        assert self.sems is not None
        popped = self.nc._tile_sem_poison_stack.pop()
        assert popped is self._sem_poison
        self.nc.clear_and_free_semaphores(list(self.sems.allocated().values()))
        self.nc.all_engine_barrier()

    TileContext._drain_and_barrier = _drain_and_barrier_split

    # Same single-wait limit applies to every lowered TPB instruction (the
    # 64B formats carry one EVENTS field). Post-process the BIR JSON before
    # walrus: hoist extra semaphore waits onto same-engine NoOps.
    import json as _json

    import concourse.bass2jax as bass2jax
    import concourse.bass_utils as bass_utils

    if getattr(bass_utils.compile_bir_kernel, "_wait_split", False):
        return

    _orig_compile = bass_utils.compile_bir_kernel

    def _split_multi_waits(bir_json):
        m = _json.loads(bir_json)
        counter = 0
        changed = False
        for fn in m["functions"]:
            for blk in fn["blocks"]:
                new_insts = []
                for inst in blk["instructions"]:
                    si = inst.get("sync_info")
                    waits = (si or {}).get("on_wait") or []
                    sem_waits = [w for w in waits if w.get("sync_type") == "semaphore"]
                    if len(waits) > 1 and len(sem_waits) == len(waits):
                        changed = True
                        for w in waits[:-1]:
                            counter += 1
                            new_insts.append({
                                "name": f"I-wsplit{counter}",
                                "opcode": "NoOp",
                                "engine": inst["engine"],
                                "ins": [],
                                "outs": [],
                                "sync_info": {"on_wait": [w], "on_update": []},
                            })
                        si["on_wait"] = [waits[-1]]
                    new_insts.append(inst)
                blk["instructions"] = new_insts
        if not changed:
            return bir_json
        return _json.dumps(m).encode()

    def _compile_bir_kernel_split(bir_json, tmpdir, neff_name="file.neff"):
        return _orig_compile(_split_multi_waits(bir_json), tmpdir, neff_name=neff_name)

    _compile_bir_kernel_split._wait_split = True
    bass_utils.compile_bir_kernel = _compile_bir_kernel_split
    bass2jax.compile_bir_kernel = _compile_bir_kernel_split


def build_kernel():
    _patch_tile_drain()
    nc = bass.Bass(target_bir_lowering=False, trn_type="TRN2")

    xT = nc.dram_tensor("xT", [C, T], F32R, kind="ExternalInput")
    wq = nc.dram_tensor("wq", [C, DG], F32R, kind="ExternalInput")
    wk = nc.dram_tensor("wk", [C, DG], F32R, kind="ExternalInput")
    wv = nc.dram_tensor("wv", [C, DG], F32R, kind="ExternalInput")
    wp = nc.dram_tensor("wp", [DG, C], F32R, kind="ExternalInput")
    bq = nc.dram_tensor("bq", [DG], F32, kind="ExternalInput")
    bk = nc.dram_tensor("bk", [DG], F32, kind="ExternalInput")
    out = nc.dram_tensor("out", [T, C], F32, kind="ExternalOutput")

    scale = 1.0 / np.sqrt(DH)

    from contextlib import ExitStack

    with tile.TileContext(nc) as tc, ExitStack() as ctx:
        from concourse.masks import make_identity

        const = ctx.enter_context(tc.tile_pool(name="const", bufs=1))
        xt_pool = ctx.enter_context(tc.tile_pool(name="xt", bufs=2))
        persist = ctx.enter_context(tc.tile_pool(name="persist", bufs=1))
        e_pool = ctx.enter_context(tc.tile_pool(name="epool", bufs=36))
        ysb_pool = ctx.enter_context(tc.tile_pool(name="ysb", bufs=5))
        yt_pool = ctx.enter_context(tc.tile_pool(name="ytp", bufs=3))
        o_pool = ctx.enter_context(tc.tile_pool(name="opool", bufs=2))
        rc_pool = ctx.enter_context(tc.tile_pool(name="rcp", bufs=3))
        ps_s = ctx.enter_context(tc.tile_pool(name="pss", bufs=3, space="PSUM"))
        ps_y = ctx.enter_context(tc.tile_pool(name="psy", bufs=4, space="PSUM"))
        ps_m = ctx.enter_context(tc.tile_pool(name="psm", bufs=1, space="PSUM"))

        # ---- constants (no DMA deps; execute at t~0) ----
        ones_bf = const.tile([128, 1], BF16)
        nc.vector.memset(ones_bf[:], 1.0)
        ident = const.tile([128, 128], BF16)
        make_identity(nc, ident)
        # trimask[k, q] = 1 where q >= k else 0 (keep lower-left of scores)
        trimask = const.tile([128, 128], BF16)
        nc.gpsimd.memset(trimask[:], 1.0)
        nc.gpsimd.affine_select(
            out=trimask[:],
            in_=trimask[:],
            compare_op=mybir.AluOpType.is_ge,
            fill=0.0,
            base=0,
            pattern=[[1, 128]],
            channel_multiplier=-1,
        )

        # ---- weight / input loads (per-ko so the first proj matmul only
        # waits on one 128-row slice of wq + xt) ----
        xT_r = xT.rearrange("(ko p) t -> p ko t", p=128)
        out_r = out.rearrange("(tt p) c -> tt p c", p=128)

        wq_sb = const.tile([128, KO, DG], F32R)
        wk_sb = const.tile([128, KO, DG], F32R)
        wv_sb = const.tile([128, KO, DG], F32R)
        wq_r = wq.rearrange("(ko p) d -> p ko d", p=128)
        wk_r = wk.rearrange("(ko p) d -> p ko d", p=128)
        wv_r = wv.rearrange("(ko p) d -> p ko d", p=128)

        _xt_tiles = {}

        def prefetch_xt(n, per_ko=False):
            if n in _xt_tiles or n >= NCHUNK:
                return
            xt = xt_pool.tile([128, KO, CHUNK], F32R, tag="xt", name=f"xt{n}")
            if per_ko:
                for ko in range(KO):
                    nc.sync.dma_start(
                        xt[:, ko, :], xT_r[:, ko, n * CHUNK:(n + 1) * CHUNK]
                    )
            else:
                nc.sync.dma_start(xt[:], xT_r[:, :, n * CHUNK:(n + 1) * CHUNK])
            _xt_tiles[n] = xt

        # interleave wq/xt0 per-ko loads for fast rampup; wk next; wv after
        xt0 = xt_pool.tile([128, KO, CHUNK], F32R, tag="xt", name="xt0")
        _xt_tiles[0] = xt0
        for ko in range(KO):
            nc.sync.dma_start(wq_sb[:, ko, :], wq_r[:, ko, :])
            nc.sync.dma_start(xt0[:, ko, :], xT_r[:, ko, 0:CHUNK])
        for ko in range(KO):
            nc.sync.dma_start(wk_sb[:, ko, :], wk_r[:, ko, :])
        bq_sb = const.tile([128, MT], F32)
        nc.sync.dma_start(bq_sb[:], bq.rearrange("(mt p) -> p mt", p=128))
        bk_sb = const.tile([128, MT], F32)
        nc.sync.dma_start(bk_sb[:], bk.rearrange("(mt p) -> p mt", p=128))
        nc.sync.dma_start(wv_sb[:], wv_r[:])
        wp_sb = const.tile([128, MT, C], F32R)
        nc.sync.dma_start(wp_sb[:], wp.rearrange("(mt p) c -> p mt c", p=128))

        # ---- persistent activations ----
        qT_sb = persist.tile([128, MT, T], BF16)     # [d_local, T] for 4 heads
        kT_sb = persist.tile([128, MT, T], BF16)
        # [tk_in, tk_tile, h, dh+1] keys-major v; the last column of each
        # head is ones so attn@v also accumulates the softmax denominator l
        v_sb = persist.tile([128, T // 128, HPG, DH + 1], BF16)
        nc.vector.memset(
            v_sb[:, :, :, DH].rearrange("p t h -> p (t h)"), 1.0
        )

        # ------------------------------------------------------------------
        # fill units: projection / transpose / output-projection work chunks
        # emitted between attention sub-steps to keep the PE busy while the
        # scalar engine runs exp.
        # ------------------------------------------------------------------
        fills = deque()  # items: (kind, callable)

        def pop_fill(k=1):
            for _ in range(k):
                if not fills:
                    return
                _, fn = fills.popleft()
                fn()

        def drain_fills(kind=None):
            """Drain until no unit of `kind` remains (FIFO, so everything
            ahead of the last such unit also runs). kind=None drains all."""
            if kind is None:
                while fills:
                    fills.popleft()[1]()
                return
            while any(kd == kind for kd, _ in fills):
                fills.popleft()[1]()

        def push_proj(n):
            """Queue projection units for chunk n (q/k into bf16 qT/kT with
            bias; v into bf16 v_sb)."""
            xt = _xt_tiles[n]
            cols = slice(n * CHUNK, (n + 1) * CHUNK)

            def qk_unit(w_sb, b_sb, dst, mt, part):
                box = {}

                def a():
                    ps = ps_m.tile([128, CHUNK], F32, tag="misc", name=f"pj{n}_{mt}")
                    box["ps"] = ps
                    for ko in range(4):
                        nc.tensor.matmul(
                            ps[:],
                            lhsT=w_sb[:, ko, mt * 128:(mt + 1) * 128],
                            rhs=xt[:, ko, :],
                            start=(ko == 0),
                            stop=False,
                        )

                def b():
                    ps = box["ps"]
                    for ko in range(4, KO):
                        nc.tensor.matmul(
                            ps[:],
                            lhsT=w_sb[:, ko, mt * 128:(mt + 1) * 128],
                            rhs=xt[:, ko, :],
                            start=False,
                            stop=(ko == KO - 1),
                        )
                    nc.vector.tensor_scalar_add(
                        dst[:, mt, cols], ps[:], b_sb[:, mt:mt + 1]
                    )

                return [(part, a), (part, b)]

            for mt in range(MT):
                fills.extend(qk_unit(wq_sb, bq_sb, qT_sb, mt, f"projq{n}"))
            for mt in range(MT):
                fills.extend(qk_unit(wk_sb, bk_sb, kT_sb, mt, f"projk{n}"))

            def v_unit(tt):
                t_tile = n * (CHUNK // 128) + tt

                def a():
                    ps = ps_m.tile([128, CHUNK], F32, tag="misc", name=f"pv{n}_{tt}")
                    for ko in range(KO):
                        nc.tensor.matmul(
                            ps[:, :DG],
                            lhsT=xt[:, ko, tt * 128:(tt + 1) * 128],
                            rhs=wv_sb[:, ko, :],
                            start=(ko == 0),
                            stop=(ko == KO - 1),
                        )
                    nc.vector.tensor_copy(v_sb[:, t_tile, :, :DH], ps[:, :DG])

                return (f"projv{n}", a)

            for tt in range(CHUNK // 128):
                fills.append(v_unit(tt))

        def push_post(n, qi, y_sb):
            """Queue transpose + output projection units for qtile qi of
            chunk n (y_sb holds normalized bf16 y [q, head, dh])."""
            t_tile = n * (CHUNK // 128) + qi
            yt = yt_pool.tile(
                [128, MT, 128], F32R, tag="yt", name=f"yt{n}_{qi}"
            )
            o_box = {}

            def transp(hp):
                def a():
                    tp = ps_m.tile(
                        [128, CHUNK], BF16, tag="misc", name=f"tp{n}_{qi}_{hp}"
                    )
                    for i2 in range(2):
                        nc.tensor.transpose(
                            tp[64 * i2:64 * i2 + 64, :128],
                            y_sb[:, 2 * hp + i2, :],
                            ident[:],
                        )
                    nc.vector.tensor_copy(yt[:, hp, :], tp[:, :128])

                return (f"post{n}", a)

            def outproj(nh):
                def a():
                    ps = ps_m.tile(
                        [128, CHUNK], F32, tag="misc", name=f"po{n}_{qi}_{nh}"
                    )
                    for mt in range(MT):
                        nc.tensor.matmul(
                            ps[:],
                            lhsT=yt[:, mt, :],
                            rhs=wp_sb[:, mt, nh * 512:(nh + 1) * 512],
                            start=(mt == 0),
                            stop=(mt == MT - 1),
                        )
                    if nh == 0:
                        o_box["o"] = o_pool.tile(
                            [128, C], F32, tag="o", name=f"o{n}_{qi}"
                        )
                    o_sb = o_box["o"]
                    nc.vector.tensor_copy(o_sb[:, nh * 512:(nh + 1) * 512], ps[:])
                    if nh == 1:
                        nc.sync.dma_start(out_r[t_tile], o_sb[:])

                return (f"post{n}", a)

            fills.append(transp(0))
            fills.append(transp(1))
            fills.append(outproj(0))
            fills.append(outproj(1))

        # ------------------------------------------------------------------
        # attention chunk: 4 head-streams; per stream a scores+exp burst
        # into SBUF e-tiles, then attn@v one (qtile) accumulation group at
        # a time into that head's PSUM bank (one open group per bank).
        # ------------------------------------------------------------------
        def attention(n):
            n_m = 4 * (n + 1)
            streams = [(p, h) for p in (0, 1) for h in (0, 1)]
            y_sbs = {}

            def emit_scores(p, h):
                es = []
                for m in range(n_m):
                    j = m - 4 * n
                    lo = 128 * j if j >= 0 else 0
                    rows = slice(64 * h, 64 * h + 64)
                    ps = ps_s.tile(
                        [128, CHUNK], F32, tag="pss", name=f"s{n}_{m}_{p}_{h}"
                    )
                    nc.tensor.matmul(
                        ps[:, lo:],
                        lhsT=kT_sb[rows, p, m * 128:(m + 1) * 128],
                        rhs=qT_sb[rows, p, n * CHUNK + lo:(n + 1) * CHUNK],
                        start=True,
                        stop=True,
                    )
                    e = e_pool.tile(
                        [128, CHUNK], BF16, tag="e", name=f"e{n}_{m}_{p}_{h}"
                    )
                    nc.scalar.activation(
                        e[:, lo:], ps[:, lo:],
                        mybir.ActivationFunctionType.Exp, scale=scale,
                    )
                    if j >= 0:
                        nc.gpsimd.tensor_mul(
                            e[:, 128 * j:128 * (j + 1)],
                            e[:, 128 * j:128 * (j + 1)],
                            trimask[:],
                        )
                    es.append(e)
                    pop_fill(1)
                return es

            def emit_avs(p, h, es):
                h_loc = 2 * p + h
                ya = ps_y.tile(
                    [128, 4, DH + 1], F32, tag="y", name=f"ya{n}_{h_loc}"
                )
                for qi in range(4):
                    m_last = 4 * n + qi
                    for m in range(m_last + 1):
                        nc.tensor.matmul(
                            ya[:, qi, :],
                            lhsT=es[m][:, 128 * qi:128 * (qi + 1)],
                            rhs=v_sb[:, m, h_loc, :],
                            start=(m == 0),
                            stop=(m == m_last),
                        )
                    # evict (qi, h_loc): fold 1/l into the PSUM read
                    recip = rc_pool.tile(
                        [128, 1], F32, tag="rc", name=f"rc{n}_{qi}_{h_loc}"
                    )
                    nc.vector.reciprocal(recip[:], ya[:, qi, DH:DH + 1])
                    if h_loc == 0:
                        y_sbs[qi] = ysb_pool.tile(
                            [128, HPG, DH], BF16, tag="ysb", name=f"ysb{n}_{qi}"
                        )
                    nc.vector.tensor_scalar_mul(
                        y_sbs[qi][:, h_loc, :], ya[:, qi, :DH], recip[:]
                    )
                    if h_loc == HPG - 1:
                        push_post(n, qi, y_sbs[qi])
                    pop_fill(1)

            prev = None
            for s in streams:
                es = emit_scores(*s)
                if prev is not None:
                    emit_avs(*prev)
                prev = (s[0], s[1], es)
            emit_avs(*prev)

        # ------------------------------------------------------------------
        # main schedule
        # ------------------------------------------------------------------
        with nc.allow_low_precision(reason="bf16 attention pipeline by design"):
            push_proj(0)
            drain_fills()          # chunk 0 projections must precede its scores
            for n in range(NCHUNK):
                prefetch_xt(n + 1)
                if n + 1 < NCHUNK:
                    push_proj(n + 1)
                # everything chunk n's scores read (qT/kT/v of chunk n) must
                # already be emitted; for n>=1 those units were queued during
                # chunk n-1 — force any stragglers out now
                for part in (f"projq{n}", f"projk{n}", f"projv{n}"):
                    drain_fills(part)
                attention(n)
            drain_fills()

    return nc


_NC_CACHE = None


def kernel(**inputs) -> np.ndarray:
    global _NC_CACHE
    x = np.asarray(inputs["x"], np.float32)
    Wq = np.asarray(inputs["Wq"], np.float32)
    Wk = np.asarray(inputs["Wk"], np.float32)
    Wv = np.asarray(inputs["Wv"], np.float32)
    Wp = np.asarray(inputs["Wp"], np.float32)
    bq = np.asarray(inputs["bq"], np.float32)
    bk = np.asarray(inputs["bk"], np.float32)
    bv = np.asarray(inputs["bv"], np.float32)
    bp = np.asarray(inputs["bp"], np.float32)

    if _NC_CACHE is None:
        _NC_CACHE = build_kernel()
    nc = _NC_CACHE

    in_maps = []
    for c in range(NCORES):
        b, g = divmod(c, GROUPS)
        rows = slice(g * DG, (g + 1) * DG)
        in_maps.append({
            "xT": np.ascontiguousarray(x[b].T),
            "wq": np.ascontiguousarray(Wq[rows, :].T),
            "wk": np.ascontiguousarray(Wk[rows, :].T),
            "wv": np.ascontiguousarray(Wv[rows, :].T),
            "wp": np.ascontiguousarray(Wp[:, rows].T),
            "bq": np.ascontiguousarray(bq[rows]),
            "bk": np.ascontiguousarray(bk[rows]),
        })

    res = run_bass_kernel_spmd(nc, in_maps, core_ids=list(range(NCORES)))

    result = np.zeros((B, T, C), np.float32)
    for c in range(NCORES):
        b = c // GROUPS
        result[b] += res.results[c]["out"]
    result += (bv @ Wp.T + bp)[None, None, :]
    return result


# revision 19
# speedup vs baseline: 1.0249x; 1.0249x over previous
"""Causal self-attention on 8 trn2 NeuronCores.

Sharding: core c -> (batch b = c // 4, head-group g = c % 4). Each core
computes 4 of the 16 heads for one batch element and the corresponding
slice of the output projection; the host sums the 4 partial projections
per batch and adds the constant bias terms (bv @ Wp.T + bp) exactly.

Device-side structure (per core):
- Projections run in fp32 (float32r full-rate PE mode); q/k are evicted
  to bf16, v to bf16.
- Scores s = k^T q run in bf16 with exact causal trimming (bf16 matmuls
  have no small-N penalty), exp runs on the scalar engine straight into
  bf16 e-tiles, and the causal mask is applied post-exp as a bf16
  triangle multiply on the gpsimd engine (PE pays nothing for masking).
- attn@v is reoriented: e is the stationary operand ([keys, q-tile],
  K=128, M=128 fully used) and v streams as moving data (N=65: dh plus
  a ones column that accumulates the softmax denominator l), which
  halves the PE time of this phase vs. the v-stationary orientation.
  y lands as [q-partition, dh | l] and normalization is a per-partition
  tensor_scalar multiply during PSUM eviction (no cross-partition
  broadcast needed).
- PSUM accumulation groups "zero regions" are a full 2 KiB bank, so
  interleaving several open accumulation groups in one bank corrupts
  them. Attention therefore runs as 4 sequential head-streams per
  chunk: a stream's scores+exp burst fills SBUF e-tiles, then its
  attn@v runs one (qtile) accumulation group at a time into that
  head's dedicated PSUM bank.
- y is transposed back to [dh, q] with PE transposes (bf16) for the
  f32r output projection.
- The attention m-loop interleaves projection / output-projection /
  transpose matmuls as filler so the PE never waits on the scalar
  engine's exp latency.
"""

from collections import deque

import numpy as np

import concourse.bass as bass
import concourse.mybir as mybir
import concourse.tile as tile
from concourse.bass_utils import run_bass_kernel_spmd

B = 2
T = 2048
C = 1024
H = 16
DH = 64
NCORES = 8
GROUPS = 4           # head groups (tensor parallel)
HPG = H // GROUPS    # heads per group = 4
DG = HPG * DH        # head-group width = 256
CHUNK = 512          # query-block size
NCHUNK = T // CHUNK  # 4
KO = C // 128        # 8 contraction subtiles for the projections
MT = DG // 128       # 2 partition tiles for qT/kT and wp rows
F32 = mybir.dt.float32
F32R = mybir.dt.float32r
BF16 = mybir.dt.bfloat16


def _patch_tile_drain():
    """This walrus build lowers Drain/NOP to a CTRL with a single sync-wait
    slot; TileContext's kernel-tail drain accumulates one wait per live
    semaphore and fails codegen. Split the waits across single-wait NOPs."""
    import bass_rust
    from concourse.tile import TileContext

    def _drain_and_barrier_split(self, tick_clock, wait_clock):
        probe = self.nc.sync.nop()
        wait_clock.add_sem_waits(
            probe.ins, tile.ScopedClock({None: tick_clock.global_clock})
        )
        waits = list(probe.ins.sync_info.on_wait or [])
        probe.ins.sync_info.on_wait = []
        # distribute the final-value waits across engines; the all-engine
        # barrier below joins them before the semaphore reset
        engines = [self.nc.sync, self.nc.tensor, self.nc.vector,
                   self.nc.scalar, self.nc.gpsimd]
        for i, w in enumerate(waits):
            n = engines[i % len(engines)].nop()
            if n.ins.sync_info is None:
                n.ins.sync_info = bass_rust.SyncInfo(on_wait=[w], on_update=[])
            else:
                n.ins.sync_info.on_wait = [w]
        self.nc.sync.drain()
        self.nc.all_engine_barrier()
        assert self.sems is not None
        popped = self.nc._tile_sem_poison_stack.pop()
        assert popped is self._sem_poison
        self.nc.clear_and_free_semaphores(list(self.sems.allocated().values()))
        self.nc.all_engine_barrier()

    TileContext._drain_and_barrier = _drain_and_barrier_split

    # Same single-wait limit applies to every lowered TPB instruction (the
    # 64B formats carry one EVENTS field). Post-process the BIR JSON before
    # walrus: hoist extra semaphore waits onto same-engine NoOps.
    import json as _json

    import concourse.bass2jax as bass2jax
    import concourse.bass_utils as bass_utils

    if getattr(bass_utils.compile_bir_kernel, "_wait_split", False):
        return

    _orig_compile = bass_utils.compile_bir_kernel

    def _split_multi_waits(bir_json):
        m = _json.loads(bir_json)
        counter = 0
        changed = False
        for fn in m["functions"]:
            for blk in fn["blocks"]:
                new_insts = []
                for inst in blk["instructions"]:
                    si = inst.get("sync_info")
                    waits = (si or {}).get("on_wait") or []
                    sem_waits = [w for w in waits if w.get("sync_type") == "semaphore"]
                    if len(waits) > 1 and len(sem_waits) == len(waits):
                        changed = True
                        for w in waits[:-1]:
                            counter += 1
                            new_insts.append({
                                "name": f"I-wsplit{counter}",
                                "opcode": "NoOp",
                                "engine": inst["engine"],
                                "ins": [],
                                "outs": [],
                                "sync_info": {"on_wait": [w], "on_update": []},
                            })
                        si["on_wait"] = [waits[-1]]
                    new_insts.append(inst)
                blk["instructions"] = new_insts
        if not changed:
            return bir_json
        return _json.dumps(m).encode()

    def _compile_bir_kernel_split(bir_json, tmpdir, neff_name="file.neff"):
        return _orig_compile(_split_multi_waits(bir_json), tmpdir, neff_name=neff_name)

    _compile_bir_kernel_split._wait_split = True
    bass_utils.compile_bir_kernel = _compile_bir_kernel_split
    bass2jax.compile_bir_kernel = _compile_bir_kernel_split


def build_kernel():
    _patch_tile_drain()
    nc = bass.Bass(target_bir_lowering=False, trn_type="TRN2")

    xT = nc.dram_tensor("xT", [C, T], F32R, kind="ExternalInput")
    wq = nc.dram_tensor("wq", [C, DG], F32R, kind="ExternalInput")
    wk = nc.dram_tensor("wk", [C, DG], F32R, kind="ExternalInput")
    wv = nc.dram_tensor("wv", [C, DG], F32R, kind="ExternalInput")
    wp = nc.dram_tensor("wp", [DG, C], F32R, kind="ExternalInput")
    bq = nc.dram_tensor("bq", [DG], F32, kind="ExternalInput")
    bk = nc.dram_tensor("bk", [DG], F32, kind="ExternalInput")
    out = nc.dram_tensor("out", [T, C], F32, kind="ExternalOutput")

    scale = 1.0 / np.sqrt(DH)

    from contextlib import ExitStack

    with tile.TileContext(nc) as tc, ExitStack() as ctx:
        from concourse.masks import make_identity

        const = ctx.enter_context(tc.tile_pool(name="const", bufs=1))
        xt_pool = ctx.enter_context(tc.tile_pool(name="xt", bufs=2))
        persist = ctx.enter_context(tc.tile_pool(name="persist", bufs=1))
        e_pool = ctx.enter_context(tc.tile_pool(name="epool", bufs=36))
        ysb_pool = ctx.enter_context(tc.tile_pool(name="ysb", bufs=5))
        yt_pool = ctx.enter_context(tc.tile_pool(name="ytp", bufs=3))
        o_pool = ctx.enter_context(tc.tile_pool(name="opool", bufs=2))
        rc_pool = ctx.enter_context(tc.tile_pool(name="rcp", bufs=3))
        ps_s = ctx.enter_context(tc.tile_pool(name="pss", bufs=2, space="PSUM"))
        ps_y = ctx.enter_context(tc.tile_pool(name="psy", bufs=4, space="PSUM"))
        ps_m = ctx.enter_context(tc.tile_pool(name="psm", bufs=2, space="PSUM"))

        # ---- constants (no DMA deps; execute at t~0) ----
        ones_bf = const.tile([128, 1], BF16)
        nc.vector.memset(ones_bf[:], 1.0)
        ident = const.tile([128, 128], BF16)
        make_identity(nc, ident)
        # trimask[k, q] = 1 where q >= k else 0 (keep lower-left of scores)
        trimask = const.tile([128, 128], BF16)
        nc.gpsimd.memset(trimask[:], 1.0)
        nc.gpsimd.affine_select(
            out=trimask[:],
            in_=trimask[:],
            compare_op=mybir.AluOpType.is_ge,
            fill=0.0,
            base=0,
            pattern=[[1, 128]],
            channel_multiplier=-1,
        )

        # ---- weight / input loads (per-ko so the first proj matmul only
        # waits on one 128-row slice of wq + xt) ----
        xT_r = xT.rearrange("(ko p) t -> p ko t", p=128)
        out_r = out.rearrange("(tt p) c -> tt p c", p=128)

        wq_sb = const.tile([128, KO, DG], F32R)
        wk_sb = const.tile([128, KO, DG], F32R)
        wv_sb = const.tile([128, KO, DG], F32R)
        wq_r = wq.rearrange("(ko p) d -> p ko d", p=128)
        wk_r = wk.rearrange("(ko p) d -> p ko d", p=128)
        wv_r = wv.rearrange("(ko p) d -> p ko d", p=128)

        _xt_tiles = {}

        def prefetch_xt(n, per_ko=False):
            if n in _xt_tiles or n >= NCHUNK:
                return
            xt = xt_pool.tile([128, KO, CHUNK], F32R, tag="xt", name=f"xt{n}")
            if per_ko:
                for ko in range(KO):
                    nc.sync.dma_start(
                        xt[:, ko, :], xT_r[:, ko, n * CHUNK:(n + 1) * CHUNK]
                    )
            else:
                nc.sync.dma_start(xt[:], xT_r[:, :, n * CHUNK:(n + 1) * CHUNK])
            _xt_tiles[n] = xt

        # interleave wq/xt0 per-ko loads for fast rampup; wk next; wv after
        xt0 = xt_pool.tile([128, KO, CHUNK], F32R, tag="xt", name="xt0")
        _xt_tiles[0] = xt0
        for ko in range(KO):
            nc.sync.dma_start(wq_sb[:, ko, :], wq_r[:, ko, :])
            nc.sync.dma_start(xt0[:, ko, :], xT_r[:, ko, 0:CHUNK])
        for ko in range(KO):
            nc.sync.dma_start(wk_sb[:, ko, :], wk_r[:, ko, :])
        bq_sb = const.tile([128, MT], F32)
        nc.sync.dma_start(bq_sb[:], bq.rearrange("(mt p) -> p mt", p=128))
        bk_sb = const.tile([128, MT], F32)
        nc.sync.dma_start(bk_sb[:], bk.rearrange("(mt p) -> p mt", p=128))
        nc.sync.dma_start(wv_sb[:], wv_r[:])
        wp_sb = const.tile([128, MT, C], F32R)
        nc.sync.dma_start(wp_sb[:], wp.rearrange("(mt p) c -> p mt c", p=128))

        # ---- persistent activations ----
        qT_sb = persist.tile([128, MT, T], BF16)     # [d_local, T] for 4 heads
        kT_sb = persist.tile([128, MT, T], BF16)
        # [tk_in, tk_tile, h, dh+1] keys-major v; the last column of each
        # head is ones so attn@v also accumulates the softmax denominator l
        v_sb = persist.tile([128, T // 128, HPG, DH + 1], BF16)
        nc.vector.memset(
            v_sb[:, :, :, DH].rearrange("p t h -> p (t h)"), 1.0
        )

        # ------------------------------------------------------------------
        # fill units: projection / transpose / output-projection work chunks
        # emitted between attention sub-steps to keep the PE busy while the
        # scalar engine runs exp.
        # ------------------------------------------------------------------
        fills = deque()  # items: (kind, callable)

        def pop_fill(k=1):
            for _ in range(k):
                if not fills:
                    return
                _, fn = fills.popleft()
                fn()

        def drain_fills(kind=None):
            """kind=None: drain everything FIFO. Otherwise run only units of
            `kind` (in order), leaving the rest queued — safe because every
            unit's data dependencies are emitted before it is queued."""
            if kind is None:
                while fills:
                    fills.popleft()[1]()
                return
            rest = []
            while fills:
                kd, fn = fills.popleft()
                if kd == kind:
                    fn()
                else:
                    rest.append((kd, fn))
            fills.extend(rest)

        def push_proj(n):
            """Queue projection units for chunk n (q/k into bf16 qT/kT with
            bias; v into bf16 v_sb)."""
            xt = _xt_tiles[n]
            cols = slice(n * CHUNK, (n + 1) * CHUNK)

            def qk_unit(w_sb, b_sb, dst, mt, part):
                box = {}

                def piece(k0):
                    def a():
                        if k0 == 0:
                            box["ps"] = ps_m.tile(
                                [128, CHUNK], F32, tag="misc", name=f"pj{n}_{mt}"
                            )
                        ps = box["ps"]
                        for ko in range(k0, k0 + 2):
                            nc.tensor.matmul(
                                ps[:],
                                lhsT=w_sb[:, ko, mt * 128:(mt + 1) * 128],
                                rhs=xt[:, ko, :],
                                start=(ko == 0),
                                stop=(ko == KO - 1),
                            )
                        if k0 == KO - 2:
                            nc.vector.tensor_scalar_add(
                                dst[:, mt, cols], ps[:], b_sb[:, mt:mt + 1]
                            )

                    return (part, a)

                return [piece(k0) for k0 in range(0, KO, 2)]

            for mt in range(MT):
                fills.extend(qk_unit(wq_sb, bq_sb, qT_sb, mt, f"projq{n}"))
            for mt in range(MT):
                fills.extend(qk_unit(wk_sb, bk_sb, kT_sb, mt, f"projk{n}"))

            def v_unit(tt):
                t_tile = n * (CHUNK // 128) + tt
                box = {}

                def piece(k0):
                    def a():
                        if k0 == 0:
                            box["ps"] = ps_m.tile(
                                [128, CHUNK], F32, tag="misc", name=f"pv{n}_{tt}"
                            )
                        ps = box["ps"]
                        for ko in range(k0, k0 + 4):
                            nc.tensor.matmul(
                                ps[:, :DG],
                                lhsT=xt[:, ko, tt * 128:(tt + 1) * 128],
                                rhs=wv_sb[:, ko, :],
                                start=(ko == 0),
                                stop=(ko == KO - 1),
                            )
                        if k0 == KO - 4:
                            nc.vector.tensor_copy(
                                v_sb[:, t_tile, :, :DH], ps[:, :DG]
                            )

                    return (f"projv{n}", a)

                return [piece(k0) for k0 in range(0, KO, 4)]

            for tt in range(CHUNK // 128):
                fills.extend(v_unit(tt))

        def push_post(n, qi, y_sb):
            """Queue transpose + output projection units for qtile qi of
            chunk n (y_sb holds normalized bf16 y [q, head, dh])."""
            t_tile = n * (CHUNK // 128) + qi
            yt = yt_pool.tile(
                [128, MT, 128], F32R, tag="yt", name=f"yt{n}_{qi}"
            )

            def transp(hp):
                def a():
                    tp = ps_m.tile(
                        [128, CHUNK], BF16, tag="misc", name=f"tp{n}_{qi}_{hp}"
                    )
                    for i2 in range(2):
                        nc.tensor.transpose(
                            tp[64 * i2:64 * i2 + 64, :128],
                            y_sb[:, 2 * hp + i2, :],
                            ident[:],
                        )
                    nc.vector.tensor_copy(yt[:, hp, :], tp[:, :128])

                return (f"post{n}", a)

            def outproj(nh):
                def a():
                    ps = ps_m.tile(
                        [128, CHUNK], F32, tag="misc", name=f"po{n}_{qi}_{nh}"
                    )
                    for mt in range(MT):
                        nc.tensor.matmul(
                            ps[:],
                            lhsT=yt[:, mt, :],
                            rhs=wp_sb[:, mt, nh * 512:(nh + 1) * 512],
                            start=(mt == 0),
                            stop=(mt == MT - 1),
                        )
                    o_sb = o_pool.tile(
                        [128, CHUNK], F32, tag="o", name=f"o{n}_{qi}_{nh}"
                    )
                    nc.vector.tensor_copy(o_sb[:], ps[:])
                    nc.sync.dma_start(
                        out_r[t_tile, :, nh * 512:(nh + 1) * 512], o_sb[:]
                    )

                return (f"post{n}", a)

            fills.append(transp(0))
            fills.append(transp(1))
            fills.append(outproj(0))
            fills.append(outproj(1))

        # ------------------------------------------------------------------
        # attention chunk: 4 head-streams; per stream a scores+exp burst
        # into SBUF e-tiles, then attn@v one (qtile) accumulation group at
        # a time into that head's PSUM bank (one open group per bank).
        # ------------------------------------------------------------------
        def attention(n):
            n_m = 4 * (n + 1)
            streams = [(p, h) for p in (0, 1) for h in (0, 1)]
            y_sbs = {}

            def emit_scores(p, h, gidx):
                # e-tiles of stream gidx reuse SBUF slots of stream gidx-2:
                # that stream's attn@v units must be emitted first
                if gidx >= 2:
                    drain_fills(f"av{gidx - 2}")
                es = []
                for m in range(n_m):
                    j = m - 4 * n
                    lo = 128 * j if j >= 0 else 0
                    rows = slice(64 * h, 64 * h + 64)
                    ps = ps_s.tile(
                        [128, CHUNK], F32, tag="pss", name=f"s{n}_{m}_{p}_{h}"
                    )
                    nc.tensor.matmul(
                        ps[:, lo:],
                        lhsT=kT_sb[rows, p, m * 128:(m + 1) * 128],
                        rhs=qT_sb[rows, p, n * CHUNK + lo:(n + 1) * CHUNK],
                        start=True,
                        stop=True,
                    )
                    e = e_pool.tile(
                        [128, CHUNK], BF16, tag="e", name=f"e{n}_{m}_{p}_{h}"
                    )
                    nc.scalar.activation(
                        e[:, lo:], ps[:, lo:],
                        mybir.ActivationFunctionType.Exp, scale=scale,
                    )
                    if j >= 0:
                        nc.gpsimd.tensor_mul(
                            e[:, 128 * j:128 * (j + 1)],
                            e[:, 128 * j:128 * (j + 1)],
                            trimask[:],
                        )
                    es.append(e)
                    pop_fill(1)
                return es

            def push_avs(p, h, es, gidx):
                h_loc = 2 * p + h
                box = {}

                def unit(qi):
                    def a():
                        if qi == 0:
                            box["ya"] = ps_y.tile(
                                [128, 4, DH + 1], F32, tag="y",
                                name=f"ya{n}_{h_loc}",
                            )
                        ya = box["ya"]
                        m_last = 4 * n + qi
                        for m in range(m_last + 1):
                            nc.tensor.matmul(
                                ya[:, qi, :],
                                lhsT=es[m][:, 128 * qi:128 * (qi + 1)],
                                rhs=v_sb[:, m, h_loc, :],
                                start=(m == 0),
                                stop=(m == m_last),
                            )
                        # evict (qi, h_loc): fold 1/l into the PSUM read
                        recip = rc_pool.tile(
                            [128, 1], F32, tag="rc", name=f"rc{n}_{qi}_{h_loc}"
                        )
                        nc.vector.reciprocal(recip[:], ya[:, qi, DH:DH + 1])
                        if h_loc == 0:
                            y_sbs[qi] = ysb_pool.tile(
                                [128, HPG, DH], BF16, tag="ysb",
                                name=f"ysb{n}_{qi}",
                            )
                        nc.vector.tensor_scalar_mul(
                            y_sbs[qi][:, h_loc, :], ya[:, qi, :DH], recip[:]
                        )
                        if h_loc == HPG - 1:
                            push_post(n, qi, y_sbs[qi])

                    return (f"av{gidx}", a)

                for qi in range(4):
                    fills.append(unit(qi))

            for s_idx, (p, h) in enumerate(streams):
                gidx = 4 * n + s_idx
                es = emit_scores(p, h, gidx)
                push_avs(p, h, es, gidx)

        # ------------------------------------------------------------------
        # main schedule
        # ------------------------------------------------------------------
        with nc.allow_low_precision(reason="bf16 attention pipeline by design"):
            push_proj(0)
            drain_fills()          # chunk 0 projections must precede its scores
            for n in range(NCHUNK):
                prefetch_xt(n + 1)
                if n + 1 < NCHUNK:
                    push_proj(n + 1)
                # everything chunk n's scores read (qT/kT/v of chunk n) must
                # already be emitted; for n>=1 those units were queued during
                # chunk n-1 — force any stragglers out now
                for part in (f"projq{n}", f"projk{n}", f"projv{n}"):
                    drain_fills(part)
                attention(n)
            drain_fills()

    return nc


_NC_CACHE = None


def kernel(**inputs) -> np.ndarray:
    global _NC_CACHE
    x = np.asarray(inputs["x"], np.float32)
    Wq = np.asarray(inputs["Wq"], np.float32)
    Wk = np.asarray(inputs["Wk"], np.float32)
    Wv = np.asarray(inputs["Wv"], np.float32)
    Wp = np.asarray(inputs["Wp"], np.float32)
    bq = np.asarray(inputs["bq"], np.float32)
    bk = np.asarray(inputs["bk"], np.float32)
    bv = np.asarray(inputs["bv"], np.float32)
    bp = np.asarray(inputs["bp"], np.float32)

    if _NC_CACHE is None:
        _NC_CACHE = build_kernel()
    nc = _NC_CACHE

    in_maps = []
    for c in range(NCORES):
        b, g = divmod(c, GROUPS)
        rows = slice(g * DG, (g + 1) * DG)
        in_maps.append({
            "xT": np.ascontiguousarray(x[b].T),
            "wq": np.ascontiguousarray(Wq[rows, :].T),
            "wk": np.ascontiguousarray(Wk[rows, :].T),
            "wv": np.ascontiguousarray(Wv[rows, :].T),
            "wp": np.ascontiguousarray(Wp[:, rows].T),
            "bq": np.ascontiguousarray(bq[rows]),
            "bk": np.ascontiguousarray(bk[rows]),
        })

    res = run_bass_kernel_spmd(nc, in_maps, core_ids=list(range(NCORES)))

    result = np.zeros((B, T, C), np.float32)
    for c in range(NCORES):
        b = c // GROUPS
        result[b] += res.results[c]["out"]
    result += (bv @ Wp.T + bp)[None, None, :]
    return result


# revision 26
# speedup vs baseline: 1.0448x; 1.0195x over previous
"""Causal self-attention on 8 trn2 NeuronCores.

Sharding: core c -> (batch b = c // 4, head-group g = c % 4). Each core
computes 4 of the 16 heads for one batch element and the corresponding
slice of the output projection; the host sums the 4 partial projections
per batch and adds the constant bias terms (bv @ Wp.T + bp) exactly.

Device-side structure (per core):
- Projections run in fp32 (float32r full-rate PE mode); q/k are evicted
  to bf16, v to bf16.
- Scores s = k^T q run in bf16 with exact causal trimming (bf16 matmuls
  have no small-N penalty), exp runs on the scalar engine straight into
  bf16 e-tiles, and the causal mask is applied post-exp as a bf16
  triangle multiply on the gpsimd engine (PE pays nothing for masking).
- attn@v is reoriented: e is the stationary operand ([keys, q-tile],
  K=128, M=128 fully used) and v streams as moving data (N=65: dh plus
  a ones column that accumulates the softmax denominator l), which
  halves the PE time of this phase vs. the v-stationary orientation.
  y lands as [q-partition, dh | l] and normalization is a per-partition
  tensor_scalar multiply during PSUM eviction (no cross-partition
  broadcast needed).
- PSUM accumulation groups "zero regions" are a full 2 KiB bank, so
  interleaving several open accumulation groups in one bank corrupts
  them. Attention therefore runs as 4 sequential head-streams per
  chunk: a stream's scores+exp burst fills SBUF e-tiles, then its
  attn@v runs one (qtile) accumulation group at a time into that
  head's dedicated PSUM bank.
- y is transposed back to [dh, q] with PE transposes (bf16) for the
  f32r output projection.
- The attention m-loop interleaves projection / output-projection /
  transpose matmuls as filler so the PE never waits on the scalar
  engine's exp latency.
"""

from collections import deque

import numpy as np

import concourse.bass as bass
import concourse.mybir as mybir
import concourse.tile as tile
from concourse.bass_utils import run_bass_kernel_spmd

B = 2
T = 2048
C = 1024
H = 16
DH = 64
NCORES = 8
GROUPS = 4           # head groups (tensor parallel)
HPG = H // GROUPS    # heads per group = 4
DG = HPG * DH        # head-group width = 256
CHUNK = 512          # query-block size
NCHUNK = T // CHUNK  # 4
KO = C // 128        # 8 contraction subtiles for the projections
MT = DG // 128       # 2 partition tiles for qT/kT and wp rows
F32 = mybir.dt.float32
F32R = mybir.dt.float32r
BF16 = mybir.dt.bfloat16


def _patch_tile_drain():
    """This walrus build lowers Drain/NOP to a CTRL with a single sync-wait
    slot; TileContext's kernel-tail drain accumulates one wait per live
    semaphore and fails codegen. Split the waits across single-wait NOPs."""
    import bass_rust
    from concourse.tile import TileContext

    def _drain_and_barrier_split(self, tick_clock, wait_clock):
        probe = self.nc.sync.nop()
        wait_clock.add_sem_waits(
            probe.ins, tile.ScopedClock({None: tick_clock.global_clock})
        )
        waits = list(probe.ins.sync_info.on_wait or [])
        probe.ins.sync_info.on_wait = []
        # distribute the final-value waits across engines; the all-engine
        # barrier below joins them before the semaphore reset
        engines = [self.nc.sync, self.nc.tensor, self.nc.vector,
                   self.nc.scalar, self.nc.gpsimd]
        for i, w in enumerate(waits):
            n = engines[i % len(engines)].nop()
            if n.ins.sync_info is None:
                n.ins.sync_info = bass_rust.SyncInfo(on_wait=[w], on_update=[])
            else:
                n.ins.sync_info.on_wait = [w]
        self.nc.sync.drain()
        self.nc.all_engine_barrier()
        assert self.sems is not None
        popped = self.nc._tile_sem_poison_stack.pop()
        assert popped is self._sem_poison
        self.nc.clear_and_free_semaphores(list(self.sems.allocated().values()))
        self.nc.all_engine_barrier()

    TileContext._drain_and_barrier = _drain_and_barrier_split

    # Same single-wait limit applies to every lowered TPB instruction (the
    # 64B formats carry one EVENTS field). Post-process the BIR JSON before
    # walrus: hoist extra semaphore waits onto same-engine NoOps.
    import json as _json

    import concourse.bass2jax as bass2jax
    import concourse.bass_utils as bass_utils

    if getattr(bass_utils.compile_bir_kernel, "_wait_split", False):
        return

    _orig_compile = bass_utils.compile_bir_kernel

    def _split_multi_waits(bir_json):
        m = _json.loads(bir_json)
        counter = 0
        changed = False
        for fn in m["functions"]:
            for blk in fn["blocks"]:
                new_insts = []
                for inst in blk["instructions"]:
                    si = inst.get("sync_info")
                    waits = (si or {}).get("on_wait") or []
                    sem_waits = [w for w in waits if w.get("sync_type") == "semaphore"]
                    if len(waits) > 1 and len(sem_waits) == len(waits):
                        changed = True
                        for w in waits[:-1]:
                            counter += 1
                            new_insts.append({
                                "name": f"I-wsplit{counter}",
                                "opcode": "NoOp",
                                "engine": inst["engine"],
                                "ins": [],
                                "outs": [],
                                "sync_info": {"on_wait": [w], "on_update": []},
                            })
                        si["on_wait"] = [waits[-1]]
                    new_insts.append(inst)
                blk["instructions"] = new_insts
        if not changed:
            return bir_json
        return _json.dumps(m).encode()

    def _compile_bir_kernel_split(bir_json, tmpdir, neff_name="file.neff"):
        return _orig_compile(_split_multi_waits(bir_json), tmpdir, neff_name=neff_name)

    _compile_bir_kernel_split._wait_split = True
    bass_utils.compile_bir_kernel = _compile_bir_kernel_split
    bass2jax.compile_bir_kernel = _compile_bir_kernel_split


def build_kernel():
    _patch_tile_drain()
    nc = bass.Bass(target_bir_lowering=False, trn_type="TRN2")

    xT = nc.dram_tensor("xT", [C, T], F32R, kind="ExternalInput")
    wq = nc.dram_tensor("wq", [C, DG], F32R, kind="ExternalInput")
    wk = nc.dram_tensor("wk", [C, DG], F32R, kind="ExternalInput")
    wv = nc.dram_tensor("wv", [C, DG], F32R, kind="ExternalInput")
    wp = nc.dram_tensor("wp", [DG, C], F32R, kind="ExternalInput")
    bq = nc.dram_tensor("bq", [DG], F32, kind="ExternalInput")
    bk = nc.dram_tensor("bk", [DG], F32, kind="ExternalInput")
    out = nc.dram_tensor("out", [T, C], F32, kind="ExternalOutput")

    scale = 1.0 / np.sqrt(DH)

    from contextlib import ExitStack

    with tile.TileContext(nc) as tc, ExitStack() as ctx:
        from concourse.masks import make_identity

        const = ctx.enter_context(tc.tile_pool(name="const", bufs=1))
        xt_pool = ctx.enter_context(tc.tile_pool(name="xt", bufs=2))
        persist = ctx.enter_context(tc.tile_pool(name="persist", bufs=1))
        e_pool = ctx.enter_context(tc.tile_pool(name="epool", bufs=36))
        ysb_pool = ctx.enter_context(tc.tile_pool(name="ysb", bufs=5))
        yt_pool = ctx.enter_context(tc.tile_pool(name="ytp", bufs=6))
        o_pool = ctx.enter_context(tc.tile_pool(name="opool", bufs=2))
        rc_pool = ctx.enter_context(tc.tile_pool(name="rcp", bufs=3))
        ps_s = ctx.enter_context(tc.tile_pool(name="pss", bufs=2, space="PSUM"))
        ps_y = ctx.enter_context(tc.tile_pool(name="psy", bufs=4, space="PSUM"))
        ps_m = ctx.enter_context(tc.tile_pool(name="psm", bufs=2, space="PSUM"))

        # ---- constants (no DMA deps; execute at t~0) ----
        ones_bf = const.tile([128, 1], BF16)
        nc.vector.memset(ones_bf[:], 1.0)
        ident = const.tile([128, 128], BF16)
        make_identity(nc, ident)
        # trimask[k, q] = 1 where q >= k else 0 (keep lower-left of scores)
        trimask = const.tile([128, 128], BF16)
        nc.gpsimd.memset(trimask[:], 1.0)
        nc.gpsimd.affine_select(
            out=trimask[:],
            in_=trimask[:],
            compare_op=mybir.AluOpType.is_ge,
            fill=0.0,
            base=0,
            pattern=[[1, 128]],
            channel_multiplier=-1,
        )

        # ---- weight / input loads (per-ko so the first proj matmul only
        # waits on one 128-row slice of wq + xt) ----
        xT_r = xT.rearrange("(ko p) t -> p ko t", p=128)
        out_r = out.rearrange("(tt p) c -> tt p c", p=128)

        wq_sb = const.tile([128, KO, DG], F32R)
        wk_sb = const.tile([128, KO, DG], F32R)
        wv_sb = const.tile([128, KO, DG], F32R)
        wq_r = wq.rearrange("(ko p) d -> p ko d", p=128)
        wk_r = wk.rearrange("(ko p) d -> p ko d", p=128)
        wv_r = wv.rearrange("(ko p) d -> p ko d", p=128)

        _xt_tiles = {}

        def prefetch_xt(n, per_ko=False):
            if n in _xt_tiles or n >= NCHUNK:
                return
            xt = xt_pool.tile([128, KO, CHUNK], F32R, tag="xt", name=f"xt{n}")
            if per_ko:
                for ko in range(KO):
                    nc.sync.dma_start(
                        xt[:, ko, :], xT_r[:, ko, n * CHUNK:(n + 1) * CHUNK]
                    )
            else:
                nc.sync.dma_start(xt[:], xT_r[:, :, n * CHUNK:(n + 1) * CHUNK])
            _xt_tiles[n] = xt

        # PE warmup: keep the tensor engine's busy-streak alive from t~0 so
        # it is fully ramped (pstate) when the first projection lands, and
        # the initial DMA latency is covered.
        warm = const.tile([128, 128], BF16)
        nc.vector.memset(warm[:], 0.0)
        wps = ps_m.tile([128, CHUNK], F32, tag="misc", name="warm")
        for _ in range(24):
            nc.tensor.matmul(
                wps[:, :128], lhsT=warm[:], rhs=warm[:], start=True, stop=True
            )

        # interleave wq/xt0 per-ko loads for fast rampup (biases early — the
        # first q eviction needs them); wk/wv/wp coarse afterwards
        xt0 = xt_pool.tile([128, KO, CHUNK], F32R, tag="xt", name="xt0")
        _xt_tiles[0] = xt0
        nc.sync.dma_start(wq_sb[:, 0, :], wq_r[:, 0, :])
        nc.sync.dma_start(xt0[:, 0, :], xT_r[:, 0, 0:CHUNK])
        bq_sb = const.tile([128, MT], F32)
        nc.sync.dma_start(bq_sb[:], bq.rearrange("(mt p) -> p mt", p=128))
        bk_sb = const.tile([128, MT], F32)
        nc.sync.dma_start(bk_sb[:], bk.rearrange("(mt p) -> p mt", p=128))
        for ko in range(1, KO):
            nc.sync.dma_start(wq_sb[:, ko, :], wq_r[:, ko, :])
            nc.sync.dma_start(xt0[:, ko, :], xT_r[:, ko, 0:CHUNK])
        nc.sync.dma_start(wk_sb[:, :4, :], wk_r[:, :4, :])
        nc.sync.dma_start(wk_sb[:, 4:, :], wk_r[:, 4:, :])
        nc.sync.dma_start(wv_sb[:], wv_r[:])
        wp_sb = const.tile([128, MT, C], F32R)
        nc.sync.dma_start(wp_sb[:], wp.rearrange("(mt p) c -> p mt c", p=128))

        # ---- persistent activations ----
        qT_sb = persist.tile([128, MT, T], BF16)     # [d_local, T] for 4 heads
        kT_sb = persist.tile([128, MT, T], BF16)
        # [tk_in, tk_tile, h, dh+1] keys-major v; the last column of each
        # head is ones so attn@v also accumulates the softmax denominator l
        v_sb = persist.tile([128, T // 128, HPG, DH + 1], BF16)
        nc.vector.memset(
            v_sb[:, :, :, DH].rearrange("p t h -> p (t h)"), 1.0
        )

        # ------------------------------------------------------------------
        # fill units: projection / transpose / output-projection work chunks
        # emitted between attention sub-steps to keep the PE busy while the
        # scalar engine runs exp.
        # ------------------------------------------------------------------
        fills = deque()  # items: (kind, callable)

        def pop_fill(k=1):
            for _ in range(k):
                if not fills:
                    return
                _, fn = fills.popleft()
                fn()

        def drain_fills(kind=None):
            """kind=None: drain everything FIFO. Otherwise run only units of
            `kind` (in order), leaving the rest queued — safe because every
            unit's data dependencies are emitted before it is queued."""
            if kind is None:
                while fills:
                    fills.popleft()[1]()
                return
            rest = []
            while fills:
                kd, fn = fills.popleft()
                if kd == kind:
                    fn()
                else:
                    rest.append((kd, fn))
            fills.extend(rest)

        def push_proj(n):
            """Queue projection units for chunk n (q/k into bf16 qT/kT with
            bias; v into bf16 v_sb)."""
            xt = _xt_tiles[n]
            cols = slice(n * CHUNK, (n + 1) * CHUNK)

            def qk_unit(w_sb, b_sb, dst, mt, part):
                box = {}

                def piece(k0):
                    def a():
                        if k0 == 0:
                            box["ps"] = ps_m.tile(
                                [128, CHUNK], F32, tag="misc", name=f"pj{n}_{mt}"
                            )
                        ps = box["ps"]
                        for ko in range(k0, k0 + 2):
                            nc.tensor.matmul(
                                ps[:],
                                lhsT=w_sb[:, ko, mt * 128:(mt + 1) * 128],
                                rhs=xt[:, ko, :],
                                start=(ko == 0),
                                stop=(ko == KO - 1),
                            )
                        if k0 == KO - 2:
                            nc.vector.tensor_scalar_add(
                                dst[:, mt, cols], ps[:], b_sb[:, mt:mt + 1]
                            )

                    return (part, a)

                return [piece(k0) for k0 in range(0, KO, 2)]

            for mt in range(MT):
                fills.extend(qk_unit(wq_sb, bq_sb, qT_sb, mt, f"projq{n}"))
            for mt in range(MT):
                fills.extend(qk_unit(wk_sb, bk_sb, kT_sb, mt, f"projk{n}"))

            def v_unit(tt):
                t_tile = n * (CHUNK // 128) + tt
                box = {}

                def piece(k0):
                    def a():
                        if k0 == 0:
                            box["ps"] = ps_m.tile(
                                [128, CHUNK], F32, tag="misc", name=f"pv{n}_{tt}"
                            )
                        ps = box["ps"]
                        for ko in range(k0, k0 + 4):
                            nc.tensor.matmul(
                                ps[:, :DG],
                                lhsT=xt[:, ko, tt * 128:(tt + 1) * 128],
                                rhs=wv_sb[:, ko, :],
                                start=(ko == 0),
                                stop=(ko == KO - 1),
                            )
                        if k0 == KO - 4:
                            nc.vector.tensor_copy(
                                v_sb[:, t_tile, :, :DH], ps[:, :DG]
                            )

                    return (f"projv{n}", a)

                return [piece(k0) for k0 in range(0, KO, 4)]

            for tt in range(CHUNK // 128):
                fills.extend(v_unit(tt))

        pending_outproj = []

        def push_post(n, qi, y_sb):
            """Queue transpose units for qtile qi of chunk n (y_sb holds
            normalized bf16 y [q, head, dh]). The qtile's output-projection
            units are held back one qtile so the yT eviction latency hides
            behind other queued work."""
            t_tile = n * (CHUNK // 128) + qi
            yt = yt_pool.tile(
                [128, MT, 128], F32R, tag="yt", name=f"yt{n}_{qi}"
            )

            def transp(hp):
                def a():
                    tp = ps_m.tile(
                        [128, CHUNK], BF16, tag="misc", name=f"tp{n}_{qi}_{hp}"
                    )
                    for i2 in range(2):
                        nc.tensor.transpose(
                            tp[64 * i2:64 * i2 + 64, :128],
                            y_sb[:, 2 * hp + i2, :],
                            ident[:],
                        )
                    nc.vector.tensor_copy(yt[:, hp, :], tp[:, :128])

                return (f"post{n}", a)

            def outproj(nh):
                def a():
                    ps = ps_m.tile(
                        [128, CHUNK], F32, tag="misc", name=f"po{n}_{qi}_{nh}"
                    )
                    for mt in range(MT):
                        nc.tensor.matmul(
                            ps[:],
                            lhsT=yt[:, mt, :],
                            rhs=wp_sb[:, mt, nh * 512:(nh + 1) * 512],
                            start=(mt == 0),
                            stop=(mt == MT - 1),
                        )
                    o_sb = o_pool.tile(
                        [128, CHUNK], F32, tag="o", name=f"o{n}_{qi}_{nh}"
                    )
                    if nh == 0:
                        nc.scalar.copy(o_sb[:], ps[:])
                    else:
                        nc.vector.tensor_copy(o_sb[:], ps[:])
                    nc.sync.dma_start(
                        out_r[t_tile, :, nh * 512:(nh + 1) * 512], o_sb[:]
                    )

                return (f"post{n}", a)

            fills.append(transp(0))
            fills.append(transp(1))
            fills.extend(pending_outproj)
            pending_outproj.clear()
            pending_outproj.extend([outproj(0), outproj(1)])

        # ------------------------------------------------------------------
        # attention chunk: 4 head-streams; per stream a scores+exp burst
        # into SBUF e-tiles, then attn@v one (qtile) accumulation group at
        # a time into that head's PSUM bank (one open group per bank).
        # ------------------------------------------------------------------
        def attention(n):
            n_m = 4 * (n + 1)
            streams = [(p, h) for p in (0, 1) for h in (0, 1)]
            y_sbs = {}

            def emit_scores(p, h, gidx):
                # e-tiles of stream gidx reuse SBUF slots of stream gidx-2:
                # that stream's attn@v units must be emitted first
                if gidx >= 2:
                    drain_fills(f"av{gidx - 2}")
                es = []
                for m in range(n_m):
                    j = m - 4 * n
                    lo = 128 * j if j >= 0 else 0
                    rows = slice(64 * h, 64 * h + 64)
                    ps = ps_s.tile(
                        [128, CHUNK], F32, tag="pss", name=f"s{n}_{m}_{p}_{h}"
                    )
                    nc.tensor.matmul(
                        ps[:, lo:],
                        lhsT=kT_sb[rows, p, m * 128:(m + 1) * 128],
                        rhs=qT_sb[rows, p, n * CHUNK + lo:(n + 1) * CHUNK],
                        start=True,
                        stop=True,
                    )
                    e = e_pool.tile(
                        [128, CHUNK], BF16, tag="e", name=f"e{n}_{m}_{p}_{h}"
                    )
                    nc.scalar.activation(
                        e[:, lo:], ps[:, lo:],
                        mybir.ActivationFunctionType.Exp, scale=scale,
                    )
                    if j >= 0:
                        nc.gpsimd.tensor_mul(
                            e[:, 128 * j:128 * (j + 1)],
                            e[:, 128 * j:128 * (j + 1)],
                            trimask[:],
                        )
                    es.append(e)
                    pop_fill(1)
                return es

            def push_avs(p, h, es, gidx):
                h_loc = 2 * p + h
                box = {}

                def unit(qi):
                    def a():
                        if qi == 0:
                            box["ya"] = ps_y.tile(
                                [128, 4, DH + 1], F32, tag="y",
                                name=f"ya{n}_{h_loc}",
                            )
                        ya = box["ya"]
                        m_last = 4 * n + qi
                        for m in range(m_last + 1):
                            nc.tensor.matmul(
                                ya[:, qi, :],
                                lhsT=es[m][:, 128 * qi:128 * (qi + 1)],
                                rhs=v_sb[:, m, h_loc, :],
                                start=(m == 0),
                                stop=(m == m_last),
                            )
                        # evict (qi, h_loc): fold 1/l into the PSUM read
                        recip = rc_pool.tile(
                            [128, 1], F32, tag="rc", name=f"rc{n}_{qi}_{h_loc}"
                        )
                        nc.vector.reciprocal(recip[:], ya[:, qi, DH:DH + 1])
                        if h_loc == 0:
                            y_sbs[qi] = ysb_pool.tile(
                                [128, HPG, DH], BF16, tag="ysb",
                                name=f"ysb{n}_{qi}",
                            )
                        nc.vector.tensor_scalar_mul(
                            y_sbs[qi][:, h_loc, :], ya[:, qi, :DH], recip[:]
                        )
                        if h_loc == HPG - 1:
                            push_post(n, qi, y_sbs[qi])

                    return (f"av{gidx}", a)

                for qi in range(4):
                    fills.append(unit(qi))

            for s_idx, (p, h) in enumerate(streams):
                gidx = 4 * n + s_idx
                es = emit_scores(p, h, gidx)
                push_avs(p, h, es, gidx)

        # ------------------------------------------------------------------
        # main schedule
        # ------------------------------------------------------------------
        with nc.allow_low_precision(reason="bf16 attention pipeline by design"):
            push_proj(0)
            drain_fills()          # chunk 0 projections must precede its scores
            for n in range(NCHUNK):
                prefetch_xt(n + 1)
                if n + 1 < NCHUNK:
                    push_proj(n + 1)
                # everything chunk n's scores read (qT/kT/v of chunk n) must
                # already be emitted; for n>=1 those units were queued during
                # chunk n-1 — force any stragglers out now
                for part in (f"projq{n}", f"projk{n}", f"projv{n}"):
                    drain_fills(part)
                attention(n)
            while fills or pending_outproj:
                drain_fills()
                fills.extend(pending_outproj)
                pending_outproj.clear()

    return nc


_NC_CACHE = None


def kernel(**inputs) -> np.ndarray:
    global _NC_CACHE
    x = np.asarray(inputs["x"], np.float32)
    Wq = np.asarray(inputs["Wq"], np.float32)
    Wk = np.asarray(inputs["Wk"], np.float32)
    Wv = np.asarray(inputs["Wv"], np.float32)
    Wp = np.asarray(inputs["Wp"], np.float32)
    bq = np.asarray(inputs["bq"], np.float32)
    bk = np.asarray(inputs["bk"], np.float32)
    bv = np.asarray(inputs["bv"], np.float32)
    bp = np.asarray(inputs["bp"], np.float32)

    if _NC_CACHE is None:
        _NC_CACHE = build_kernel()
    nc = _NC_CACHE

    in_maps = []
    for c in range(NCORES):
        b, g = divmod(c, GROUPS)
        rows = slice(g * DG, (g + 1) * DG)
        in_maps.append({
            "xT": np.ascontiguousarray(x[b].T),
            "wq": np.ascontiguousarray(Wq[rows, :].T),
            "wk": np.ascontiguousarray(Wk[rows, :].T),
            "wv": np.ascontiguousarray(Wv[rows, :].T),
            "wp": np.ascontiguousarray(Wp[:, rows].T),
            "bq": np.ascontiguousarray(bq[rows]),
            "bk": np.ascontiguousarray(bk[rows]),
        })

    res = run_bass_kernel_spmd(nc, in_maps, core_ids=list(range(NCORES)))

    result = np.zeros((B, T, C), np.float32)
    for c in range(NCORES):
        b = c // GROUPS
        result[b] += res.results[c]["out"]
    result += (bv @ Wp.T + bp)[None, None, :]
    return result
